# revision 11
# baseline (speedup 1.0000x reference)
"""Causal self-attention (B=4, S=2048, D=1024, H=16) on 8 Trainium2 cores.

Sharding: core c = (batch b = c//2, head-group g = c%2). Each core computes
heads [8g, 8g+8) of its batch: qkv projection (column-sliced), attention, and
the output projection against its half of w_out's columns, after an AllGather
of the per-head attention outputs within the (b) pair. Output is concatenated
on the host: out[b, :, 512g:512(g+1)] = core (2b+g).

Device dataflow (per core, all matmuls fp32r = full-rate fp32):
  xT [D,S] (host pre-transposed) --+--> Q^T [512,S] = Wq.T x   (lhsT=Wq)
                                   +--> K^T [512,S] = Wk.T x   (lhsT=Wk)
                                   +--> V   [S,520] = x Wv_aug (lhsT=xT)
  Wv_aug has a zero column + bias 1.0 appended per head, so V column 64 of
  each head is identically 1 -> the PV matmul produces softmax denominators
  as attnT row 64 for free.
  Per head h: S^T[k,q] = (K^T_h)^T Q^T_h (causal block-skipped),
  P^T = exp(S^T) (ACT; scale 1/8 folded into Wq on host), diagonal 128x128
  blocks masked multiplicatively, attnT[65,S] += V_aug_h^T P^T accumulated
  over k-tiles in PSUM. Then recip of row 64 (DVE approx), gpsimd
  partition_broadcast to 64 rows, one DVE mul -> normalized attnT in SBUF.
  AllGather (pairs) of attnT [512,S] -> [1024,S]; out-proj uses gathered
  attnT tiles as the stationary operand: out[q,j] = sum_c attnT[c,q] W2[c,j],
  bias added via a K=1 ones-row matmul.
"""

import numpy as np

import concourse.bass as bass
import concourse.mybir as mybir
import concourse.tile as tile
from concourse import bacc

F32 = mybir.dt.float32
F32R = mybir.dt.float32r
EXP = mybir.ActivationFunctionType.Exp

B, S, D, H = 4, 2048, 1024, 16
HD = 64          # head dim
HPC = 8          # heads per core
CLOC = HPC * HD  # 512 local attention feature columns
VW = HPC * (HD + 1)  # 520: V width incl. per-head ones column
NT = S // 128    # 16 s-tiles
DT = D // 128    # 8 d-tiles


def build_nc(n_cores=8, debug_taps=False):
    nc = bacc.Bacc(
        "TRN2",
        target_bir_lowering=False,
        debug=False,
        num_devices=n_cores,
    )
    xT_d = nc.dram_tensor("xT", [D, S], F32R, kind="ExternalInput").ap()
    wq_d = nc.dram_tensor("wq", [D, CLOC], F32R, kind="ExternalInput").ap()
    wk_d = nc.dram_tensor("wk", [D, CLOC], F32R, kind="ExternalInput").ap()
    wv_d = nc.dram_tensor("wv", [D, VW], F32R, kind="ExternalInput").ap()
    bq_d = nc.dram_tensor("bq", [128, 4], F32, kind="ExternalInput").ap()
    bk_d = nc.dram_tensor("bk", [128, 4], F32, kind="ExternalInput").ap()
    bv_d = nc.dram_tensor("bv", [1, VW], F32R, kind="ExternalInput").ap()
    w2_d = nc.dram_tensor("w2", [D, 512], F32R, kind="ExternalInput").ap()
    bo_d = nc.dram_tensor("bout", [1, 512], F32R, kind="ExternalInput").ap()
    mk_d = nc.dram_tensor("mask", [128, 128], F32R, kind="ExternalInput").ap()
    on_d = nc.dram_tensor("ones", [1, 128], F32R, kind="ExternalInput").ap()
    out_d = nc.dram_tensor("out", [S, 512], F32, kind="ExternalOutput").ap()
    dbg = {}
    if debug_taps:
        for nm, shape in [
            ("dbg_qT0", [128, S]), ("dbg_kT0", [128, S]), ("dbg_v0", [128, VW]),
            ("dbg_pt", [128, S]), ("dbg_att", [65, S]), ("dbg_recip", [1, S]),
            ("dbg_asc0", [128, S]), ("dbg_ccpeer", [128, S]),
        ]:
            dbg[nm] = nc.dram_tensor(nm, shape, F32, kind="ExternalOutput").ap()

    groups = [[2 * i, 2 * i + 1] for i in range(n_cores // 2)]

    with tile.TileContext(nc) as tc:
        with (
            tc.tile_pool(name="const", bufs=1) as cp,
            tc.tile_pool(name="dram", bufs=1, space="DRAM") as dram,
        ):
            ones = cp.tile([1, 128], F32R)
            nc.sync.dma_start(ones[:], on_d[:])
            mask_s = cp.tile([128, 128], F32R)
            nc.sync.dma_start(mask_s[:], mk_d[:])
            bq_s = cp.tile([128, 4], F32)
            nc.sync.dma_start(bq_s[:], bq_d[:])
            bk_s = cp.tile([128, 4], F32)
            nc.sync.dma_start(bk_s[:], bk_d[:])
            bv_s = cp.tile([1, VW], F32R)
            nc.sync.dma_start(bv_s[:], bv_d[:])
            bo_s = cp.tile([1, 512], F32R)
            nc.sync.dma_start(bo_s[:], bo_d[:])

            qkvp = tc.alloc_tile_pool(name="qkv", bufs=1)
            qT = [qkvp.tile([128, S], F32R, name=f"qT{i}") for i in range(4)]
            kT = [qkvp.tile([128, S], F32R, name=f"kT{i}") for i in range(4)]
            vv = [qkvp.tile([128, VW], F32R, name=f"v{i}") for i in range(NT)]

            # ---------------- phase 1: projections ----------------
            with (
                tc.tile_pool(name="wp", bufs=1) as wp,
                tc.tile_pool(name="xp", bufs=1) as xp,
                tc.tile_pool(name="pp", bufs=1, space="PSUM") as pp,
            ):
                wq_s, wk_s, wv_s = [], [], []
                for dt in range(DT):
                    w1 = wp.tile([128, CLOC], F32R, name=f"wqs{dt}")
                    nc.sync.dma_start(w1[:], wq_d[128 * dt : 128 * dt + 128, :])
                    wq_s.append(w1)
                    w2s = wp.tile([128, CLOC], F32R, name=f"wks{dt}")
                    nc.sync.dma_start(w2s[:], wk_d[128 * dt : 128 * dt + 128, :])
                    wk_s.append(w2s)
                    w3 = wp.tile([128, VW], F32R, name=f"wvs{dt}")
                    nc.sync.dma_start(w3[:], wv_d[128 * dt : 128 * dt + 128, :])
                    wv_s.append(w3)

                for qc in range(4):
                    c0 = 512 * qc
                    xt = []
                    for dt in range(DT):
                        t = xp.tile(
                            [128, 512], F32R, name=f"xt{qc}_{dt}",
                            tag=f"xt{dt}", bufs=2,
                        )
                        nc.sync.dma_start(
                            t[:], xT_d[128 * dt : 128 * dt + 128, c0 : c0 + 512]
                        )
                        xt.append(t)
                    for ct in range(4):
                        ps_q = pp.tile([128, 512], F32, tag="pq", bufs=4,
                                       name=f"psq{qc}_{ct}")
                        for dt in range(DT):
                            nc.tensor.matmul(
                                ps_q[:],
                                wq_s[dt][:, 128 * ct : 128 * ct + 128],
                                xt[dt][:],
                                start=(dt == 0),
                                stop=(dt == DT - 1),
                            )
                        nc.vector.tensor_scalar_add(
                            qT[ct][:, c0 : c0 + 512], ps_q[:], bq_s[:, ct : ct + 1]
                        )
                        ps_k = pp.tile([128, 512], F32, tag="pq", bufs=4,
                                       name=f"psk{qc}_{ct}")
                        for dt in range(DT):
                            nc.tensor.matmul(
                                ps_k[:],
                                wk_s[dt][:, 128 * ct : 128 * ct + 128],
                                xt[dt][:],
                                start=(dt == 0),
                                stop=(dt == DT - 1),
                            )
                        nc.vector.tensor_scalar_add(
                            kT[ct][:, c0 : c0 + 512], ps_k[:], bk_s[:, ct : ct + 1]
                        )
                    for stl in range(4):
                        st = 4 * qc + stl
                        ps_v = pp.tile([128, VW], F32, tag="pv", bufs=2,
                                       name=f"psv{st}")
                        xsl = slice(128 * stl, 128 * stl + 128)
                        for dt in range(DT):
                            nc.tensor.matmul(
                                ps_v[:, 0:512],
                                xt[dt][:, xsl],
                                wv_s[dt][:, 0:512],
                                start=(dt == 0),
                                stop=False,
                            )
                            nc.tensor.matmul(
                                ps_v[:, 512:VW],
                                xt[dt][:, xsl],
                                wv_s[dt][:, 512:VW],
                                start=(dt == 0),
                                stop=False,
                            )
                        nc.tensor.matmul(
                            ps_v[:, 0:512], ones[:], bv_s[:, 0:512],
                            start=False, stop=True,
                        )
                        nc.tensor.matmul(
                            ps_v[:, 512:VW], ones[:], bv_s[:, 512:VW],
                            start=False, stop=True,
                        )
                        nc.vector.tensor_copy(vv[st][:], ps_v[:])

            if debug_taps:
                nc.sync.dma_start(dbg["dbg_qT0"][:], qT[0][:].bitcast(F32))
                nc.sync.dma_start(dbg["dbg_kT0"][:], kT[0][:].bitcast(F32))
                nc.sync.dma_start(dbg["dbg_v0"][:], vv[0][:].bitcast(F32))

            # ---------------- phase 2: attention ----------------
            with (
                tc.tile_pool(name="asb", bufs=1) as asb,
                tc.tile_pool(name="ptp", bufs=1) as ptp,
                tc.tile_pool(name="bcp", bufs=1) as bcp,
                tc.tile_pool(name="pa", bufs=1, space="PSUM") as pa,
            ):
                attnT_sc = [asb.tile([128, S], F32R, name=f"attnTsc{i}")
                            for i in range(4)]
                for hl in range(HPC):
                    ct_h = hl // 2
                    rb = 64 * (hl % 2)
                    qTh = qT[ct_h][rb : rb + 64, :]
                    kTh = kT[ct_h][rb : rb + 64, :]
                    at_ps = pa.tile([65, S], F32, tag="attnT", bufs=1,
                                    name=f"atps{hl}")
                    for ki in range(NT):
                        q0 = 128 * ki
                        pTt = ptp.tile([128, S], F32R, tag="pt", bufs=2,
                                       name=f"pt{hl}_{ki}")
                        for win in range(2):
                            w0, w1 = 1024 * win, 1024 * win + 1024
                            if w1 <= q0:
                                continue
                            lo = max(q0, w0)
                            ps_s = pa.tile([128, 1024], F32, tag="st", bufs=2,
                                           name=f"pss{hl}_{ki}_{win}")
                            cstart = lo
                            while cstart < w1:
                                cend = min((cstart // 512 + 1) * 512, w1)
                                nc.tensor.matmul(
                                    ps_s[:, cstart - w0 : cend - w0],
                                    kTh[:, q0 : q0 + 128],
                                    qTh[:, cstart:cend],
                                    start=True,
                                    stop=True,
                                )
                                cstart = cend
                            nc.scalar.activation(
                                pTt[:, lo:w1], ps_s[:, lo - w0 : 1024], EXP
                            )
                        nc.vector.tensor_mul(
                            pTt[:, q0 : q0 + 128],
                            pTt[:, q0 : q0 + 128],
                            mask_s[:],
                        )
                        if debug_taps and hl == 0 and ki == 0:
                            dpt = asb.tile([128, S], F32, name="dpt")
                            nc.vector.tensor_copy(dpt[:], pTt[:])
                            nc.sync.dma_start(dbg["dbg_pt"][:], dpt[:])
                        vh = vv[ki][:, (HD + 1) * hl : (HD + 1) * hl + 65]
                        for qj in range(ki // 4, 4):
                            qstart = max(512 * qj, q0)
                            n = 512 * qj + 512 - qstart
                            nc.tensor.matmul(
                                at_ps[0:65, qstart : qstart + n],
                                vh,
                                pTt[:, qstart : qstart + n],
                                start=(ki == 0),
                                stop=(ki == 4 * qj + 3),
                            )
                    if debug_taps and hl == 0:
                        datt = asb.tile([65, S], F32, name="datt")
                        nc.vector.tensor_copy(datt[:], at_ps[0:65, :])
                        nc.sync.dma_start(dbg["dbg_att"][:], datt[:])
                    den = bcp.tile([1, S], F32, tag="den", bufs=1,
                                   name=f"den{hl}")
                    nc.vector.tensor_copy(den[:], at_ps[64:65, :])
                    recip = bcp.tile([1, S], F32, tag="recip", bufs=1,
                                     name=f"recip{hl}")
                    nc.vector.reciprocal_approx_fast(
                        out=recip[:], in_=den[:]
                    )
                    bcast = bcp.tile([64, S], F32, tag="bcast", bufs=1,
                                     name=f"bcast{hl}")
                    nc.gpsimd.partition_broadcast(bcast[:], recip[:], channels=64)
                    if debug_taps and hl == 0:
                        nc.sync.dma_start(dbg["dbg_recip"][:], recip[:])
                    nc.vector.tensor_mul(
                        attnT_sc[ct_h][rb : rb + 64, :],
                        at_ps[0:64, :],
                        bcast[:],
                    )

                # ship local attnT to DRAM for the pair AllGather
                cc_in = dram.tile([CLOC, S], F32R)
                for t in range(4):
                    nc.sync.dma_start(
                        cc_in[128 * t : 128 * t + 128, :], attnT_sc[t][:]
                    )

            if debug_taps:
                nc.sync.dma_start(dbg["dbg_asc0"][:], attnT_sc[0][:].bitcast(F32))
            qkvp.release()
            cc_out = dram.tile([2 * CLOC, S], F32R)
            nc.gpsimd.collective_compute(
                "AllGather",
                mybir.AluOpType.bypass,
                replica_groups=groups,
                ins=[cc_in.opt()],
                outs=[cc_out.opt()],
            )

            if debug_taps:
                nc.sync.dma_start(dbg["dbg_ccpeer"][:], cc_out[512:640, :].bitcast(F32))

            # ---------------- phase 3: output projection ----------------
            with (
                tc.tile_pool(name="op", bufs=1) as op,
                tc.tile_pool(name="po", bufs=1, space="PSUM") as po,
            ):
                w2_s, ag = [], []
                for ct in range(8):
                    w = op.tile([128, 512], F32R, name=f"w2s{ct}")
                    nc.sync.dma_start(w[:], w2_d[128 * ct : 128 * ct + 128, :])
                    w2_s.append(w)
                    a = op.tile([128, S], F32R, name=f"ag{ct}")
                    nc.sync.dma_start(
                        a[:], cc_out[128 * ct : 128 * ct + 128, :]
                    )
                    ag.append(a)
                for qt in range(NT):
                    ps_o = po.tile([128, 512], F32, tag="po", bufs=4,
                                   name=f"pso{qt}")
                    for ct in range(8):
                        nc.tensor.matmul(
                            ps_o[:],
                            ag[ct][:, 128 * qt : 128 * qt + 128],
                            w2_s[ct][:],
                            start=(ct == 0),
                            stop=False,
                        )
                    nc.tensor.matmul(
                        ps_o[:], ones[:], bo_s[:], start=False, stop=True
                    )
                    o_sb = op.tile([128, 512], F32, tag="osb", bufs=4,
                                   name=f"osb{qt}")
                    nc.scalar.copy(o_sb[:], ps_o[:])
                    nc.sync.dma_start(
                        out_d[128 * qt : 128 * qt + 128, :], o_sb[:]
                    )

    nc.compile()
    return nc


def shard_inputs(x, w_qkv, b_qkv, w_out, b_out, n_cores=8):
    x = np.asarray(x, dtype=np.float32)
    w_qkv = np.asarray(w_qkv, dtype=np.float32)
    b_qkv = np.asarray(b_qkv, dtype=np.float32)
    w_out = np.asarray(w_out, dtype=np.float32)
    b_out = np.asarray(b_out, dtype=np.float32)
    mask = np.triu(np.ones((128, 128), dtype=np.float32))
    per_core = []
    for c in range(n_cores):
        b, g = divmod(c, 2)
        wv = np.zeros((D, VW), np.float32)
        bv = np.zeros((1, VW), np.float32)
        for hl in range(HPC):
            src = 2 * D + CLOC * g + HD * hl
            wv[:, (HD + 1) * hl : (HD + 1) * hl + HD] = w_qkv[:, src : src + HD]
            bv[0, (HD + 1) * hl : (HD + 1) * hl + HD] = b_qkv[src : src + HD]
            bv[0, (HD + 1) * hl + HD] = 1.0
        per_core.append(
            {
                "xT": np.ascontiguousarray(x[b].T),
                "wq": np.ascontiguousarray(w_qkv[:, CLOC * g : CLOC * g + CLOC])
                / 8.0,
                "wk": np.ascontiguousarray(
                    w_qkv[:, D + CLOC * g : D + CLOC * g + CLOC]
                ),
                "wv": wv,
                "bq": np.ascontiguousarray(
                    (b_qkv[CLOC * g : CLOC * g + CLOC] / 8.0).reshape(4, 128).T
                ),
                "bk": np.ascontiguousarray(
                    b_qkv[D + CLOC * g : D + CLOC * g + CLOC].reshape(4, 128).T
                ),
                "bv": bv,
                "w2": np.ascontiguousarray(w_out[:, 512 * g : 512 * g + 512]),
                "bout": np.ascontiguousarray(b_out[512 * g : 512 * g + 512])
                .reshape(1, 512),
                "mask": mask,
                "ones": np.ones((1, 128), np.float32),
            }
        )
    return per_core


def unshard_outputs(results, n_cores=8):
    out = np.empty((B, S, D), dtype=np.float32)
    for c in range(n_cores):
        b, g = divmod(c, 2)
        out[b, :, 512 * g : 512 * g + 512] = results[c]["out"]
    return out


_NC_CACHE = {}


def get_nc(n_cores=8):
    if n_cores not in _NC_CACHE:
        _NC_CACHE[n_cores] = build_nc(n_cores)
    return _NC_CACHE[n_cores]


def kernel(x, w_qkv, b_qkv, w_out, b_out):
    from concourse.bass_utils import run_bass_kernel_spmd

    nc = get_nc(8)
    in_maps = shard_inputs(x, w_qkv, b_qkv, w_out, b_out, 8)
    res = run_bass_kernel_spmd(nc, in_maps, list(range(8)))
    return unshard_outputs(res.results, 8)
